# revision 1
# baseline (speedup 1.0000x reference)
"""Trainium2 Bass kernel for 3-layer hetero-GNN message passing (RGCN-style).

Reference semantics (per layer l):
    agg_ss = segment_sum(hs[ss_src], ss_dst) / max(indeg_ss, 1)
    agg_ds = segment_sum(hd[ds_src], ds_dst) / max(indeg_ds, 1)
    hs     = relu(agg_ss @ W_rel[l,0] + agg_ds @ W_rel[l,1] + hs @ W_loop[l] + bias[l])
(doc features hd never change, so agg_ds is layer-invariant.)

Sharding: destination ("sentence") nodes are bin-packed on the host into
128-node bins balanced by in-degree; 98 bins per core x 8 cores.  Edges are
partitioned by destination bin.  Per-relation weights are replicated.  Each
layer ends with an AllGather of the updated node features so every core has
the full "halo" table for the next layer's source gathers (edge-cut
partitioning with full replication of the gather table).

Device inner loop per bin (uniform across cores -> single SPMD program):
  - one indirect DMA gathers the bin's (padded) 128*NCH source rows
  - a fused DVE is_equal builds the one-hot edge->lane matrix S from
    precomputed lane ids
  - NCH matmuls accumulate aggT = sum_e G_e^T S_e in PSUM ([H, dst] layout)
  - aggT * (1/deg) , then a 4-matmul PSUM chain forms
    agg_ss@W0 + agg_ds@W1 + hs@Wl + bias, ACT applies ReLU
  - result written back ([dst,H] for the table / AllGather shard and
    [H,dst] transposed for the next layer's self-loop lhsT)
"""

import os
import sys
import heapq

import numpy as np

for _p in ("/opt/trn_rl_repo", "/root/.axon_site/_ro/trn_rl_repo"):
    if os.path.isdir(_p) and _p not in sys.path:
        sys.path.insert(0, _p)

P = 128
H = 128


class Cfg:
    def __init__(self, ncores, nbins_core, ns, nd, nlayers, nch_ss, nch_ds,
                 nq=4, sb_sizes=(), bf16_tables=False):
        self.NCORES = ncores
        self.NBINS = nbins_core              # bins per core
        self.SLOTS_CORE = nbins_core * P
        self.SLOTS_TOTAL = self.SLOTS_CORE * ncores
        self.NS = ns
        self.ND = nd
        self.L = nlayers
        self.NCH_SS = nch_ss
        self.NCH_DS = nch_ds
        self.NQ = nq                         # src quartiles for int16 gather
        self.QSIZE = self.SLOTS_TOTAL // nq
        self.SB = list(sb_sizes)             # superbin widths (all 4)
        self.BF16 = bf16_tables


def pack_bins(deg, nbins, cap=P):
    """LPT bin packing: assign nodes to bins balancing total degree with a
    node-count cap per bin.  Returns per-bin node lists."""
    order = np.argsort(-deg, kind="stable")
    counts = np.zeros(nbins, np.int64)
    loads = np.zeros(nbins, np.int64)
    bins = [[] for _ in range(nbins)]
    heap = [(0, b) for b in range(nbins)]
    heapq.heapify(heap)
    for n in order:
        while True:
            _, b = heapq.heappop(heap)
            if counts[b] < cap:
                break
        bins[b].append(int(n))
        counts[b] += 1
        loads[b] += int(deg[n])
        if counts[b] < cap:
            heapq.heappush(heap, (int(loads[b]), b))
    return bins, loads


def pack_bins_q(qvec, cls, nbins, nq=4, cap=P, ncand=6):
    """Quartile-aware LPT: balance per-(bin, src-class) in-edge loads.

    qvec [ns, nq]: node's in-edge count per source class.
    cls  [ns]: node's own class (must get a lane == cls mod nq in its bin).
    Greedy on max class load, with per-bin class capacity cap//nq.
    Returns per-bin node lists (each list entry (node, cls))."""
    tot = qvec.sum(1)
    order = np.argsort(-tot, kind="stable")
    ccap = cap // nq
    counts = np.zeros(nbins, np.int64)
    ccounts = np.zeros((nbins, nq), np.int64)
    qloads = np.zeros((nbins, nq), np.int64)
    key = np.zeros(nbins, np.int64)
    bins = [[] for _ in range(nbins)]
    heap = [(0, b) for b in range(nbins)]
    heapq.heapify(heap)
    for n in order:
        c = int(cls[n])
        v = qvec[n]
        cands, stash = [], []
        while heap and len(cands) < ncand:
            k, b = heapq.heappop(heap)
            if k != key[b] or counts[b] >= cap:
                continue  # stale or permanently full
            if ccounts[b, c] >= ccap:
                stash.append((k, b))  # full for this class only
                continue
            cands.append(b)
        assert cands, "no bin with free class slot"
        best = min(cands, key=lambda b: (int(np.max(qloads[b] + v)),
                                         int(qloads[b].sum())))
        bins[best].append(int(n))
        counts[best] += 1
        ccounts[best, c] += 1
        qloads[best] += v
        key[best] = int(qloads[best].max())
        for k, b in stash:
            heapq.heappush(heap, (k, b))
        for b in cands:
            if counts[b] < cap:
                heapq.heappush(heap, (int(key[b]), b))
    return bins, qloads


def _edge_meta(dst_slot, src_idx, nbins_total, nch, nq, qsize,
               interleaved=False):
    """Per-(bin, src-quartile) edge arrays for dma_gather.

    Edges of a bin are grouped by source quartile (interleaved: q = src % nq,
    local row = src // nq; else q = src // qsize, local = src % qsize);
    within a group edge k lands at partition k%128, chunk k//128 (dma_gather
    layout).
    Returns (idx16, lane, glob):
      idx16 [nbins, nq, 128, nch*8] int16  -- local row id, wrapped
          16-partition layout replicated to 128 partitions; pad 0
      lane  [nbins, nq, 128, nch] float32  -- dst lane in bin; pad -1
      glob  [nbins, nq, nch*128] int64     -- global src row id (pads map to
          a valid row of the right quartile)
    """
    if interleaved:
        q_of_edge = src_idx % nq
        src_local = (src_idx // nq).astype(np.int16)
    else:
        q_of_edge = src_idx // qsize
        src_local = (src_idx % qsize).astype(np.int16)
    bin_of_edge = dst_slot // P
    lane_of_edge = (dst_slot % P).astype(np.float32)
    group = bin_of_edge * nq + q_of_edge
    order = np.argsort(group, kind="stable")
    g_sorted = group[order]
    src_sorted = src_local[order]
    lane_sorted = lane_of_edge[order]
    ngroups = nbins_total * nq
    counts = np.bincount(g_sorted, minlength=ngroups)
    starts = np.concatenate([[0], np.cumsum(counts)[:-1]])
    k_in_g = np.arange(len(order)) - starts[g_sorted]
    assert k_in_g.max(initial=0) < nch * P, (k_in_g.max(initial=0), nch * P)
    pp = (k_in_g % P).astype(np.int64)
    cc = (k_in_g // P).astype(np.int64)

    lane = np.full((ngroups, P, nch), -1.0, np.float32)
    lane[g_sorted, pp, cc] = lane_sorted

    # idx in dma_gather wrapped layout: idx of edge i sits at
    # [i % 16, i // 16] of a [16, nch*8] array, replicated to 128 partitions.
    idx_flat = np.zeros((ngroups, nch * P), np.int16)
    idx_flat[g_sorted, k_in_g] = src_sorted
    glob = idx_flat.astype(np.int64).reshape(nbins_total, nq, nch * P)
    qbase = np.arange(nq, dtype=np.int64)
    if interleaved:
        glob = glob * nq + qbase[None, :, None]
    else:
        glob = glob + (qbase * qsize)[None, :, None]
    idx16 = idx_flat.reshape(ngroups, nch * 8, 16).transpose(0, 2, 1)
    idx16 = np.broadcast_to(idx16[:, None, :, :], (ngroups, 8, 16, nch * 8))
    idx16 = idx16.reshape(ngroups, P, nch * 8)
    return (np.ascontiguousarray(idx16.reshape(nbins_total, nq, P, nch * 8)),
            np.ascontiguousarray(lane.reshape(nbins_total, nq, P, nch)),
            glob)


def _pack_meta_superbins(idx16, lane, sb_sizes):
    """Pack meta per superbin (group of consecutive bins).

    For a superbin of w bins: per quartile the w bins' idx streams concatenate
    (each bin is a whole number of 128-edge chunks, so wrapped int16 layouts
    concatenate along the free axis).

    Returns list over superbins of int32 arrays
      [P, nq*w*nch (lanes f32) + nq*w*nch*4 (idx16)]
    """
    nbins, nq, _, nch8 = idx16.shape
    nch = nch8 // 8
    out = []
    b0 = 0
    for w in sb_sizes:
        # lanes: [w, nq, P, nch] -> [P, nq, w, nch] -> [P, nq*w*nch]
        lane_sb = lane[b0:b0 + w].transpose(2, 1, 0, 3).reshape(P, nq * w * nch)
        lane_i32 = np.ascontiguousarray(lane_sb).view(np.int32)
        # idx16: [w, nq, P, nch8] -> [P, nq, w, nch8] -> [P, nq*w*nch8] int16
        idx_sb = idx16[b0:b0 + w].transpose(2, 1, 0, 3).reshape(P, nq * w * nch8)
        idx_i32 = np.ascontiguousarray(idx_sb).view(np.int32)
        out.append(np.ascontiguousarray(
            np.concatenate([lane_i32, idx_i32], axis=1)))
        b0 += w
    return out


def preprocess(inputs, ncores=8, nbins_core=None):
    """Host-side graph partitioning.  Returns (cfg, per-core input maps,
    slot2node) -- slot2node maps device output rows back to node ids."""
    s_feat = np.asarray(inputs["s_feat"], np.float32)
    doc_feat = np.asarray(inputs["doc_feat"], np.float32)
    W_rel = np.asarray(inputs["W_rel"], np.float32)
    W_loop = np.asarray(inputs["W_loop"], np.float32)
    bias = np.asarray(inputs["bias"], np.float32)
    ss_src = np.asarray(inputs["ss_src"], np.int64)
    ss_dst = np.asarray(inputs["ss_dst"], np.int64)
    ds_src = np.asarray(inputs["ds_src"], np.int64)
    ds_dst = np.asarray(inputs["ds_dst"], np.int64)

    ns, h = s_feat.shape
    nd = doc_feat.shape[0]
    nlayers = W_loop.shape[0]
    assert h == H

    if nbins_core is None:
        nbins_core = int(np.ceil(ns / (ncores * P)))
    nbins_core = (nbins_core + 3) // 4 * 4  # whole superbins of 4
    nbins_total = nbins_core * ncores
    slots_core = nbins_core * P
    slots_total = nbins_total * P

    cnt_ss = np.bincount(ss_dst, minlength=ns)
    cnt_ds = np.bincount(ds_dst, minlength=ns)
    deg_ss = np.maximum(cnt_ss, 1).astype(np.float32)
    deg_ds = np.maximum(cnt_ds, 1).astype(np.float32)

    # src quartiles: int16 gather indices need local row < 32768.
    # quartile = slot % 4 (interleaved sub-tables via elem_step), and a
    # node's class (= its slot % 4) is fixed upfront as node_id % 4 so the
    # packer can balance per-(bin, src-class) edge loads directly.
    NQ = 4
    qsize = slots_total // NQ
    assert slots_total % NQ == 0 and qsize <= 32767

    cls = (np.arange(ns) % NQ).astype(np.int64)
    qvec = np.zeros((ns, NQ), np.int64)
    np.add.at(qvec, (ss_dst, cls[ss_src]), 1)
    bins, _qloads = pack_bins_q(qvec, cls, nbins_total, nq=NQ)

    slot2node = np.full(slots_total, -1, np.int64)
    node2slot = np.full(ns, -1, np.int64)
    for b, nodes in enumerate(bins):
        nxt = [r for r in range(NQ)]
        for n in nodes:
            r = int(cls[n])
            lane_i = nxt[r]
            nxt[r] += NQ
            s = b * P + lane_i
            slot2node[s] = n
            node2slot[n] = s
    assert (node2slot >= 0).all()

    ss_dst_slot = node2slot[ss_dst]
    ss_src_slot = node2slot[ss_src]
    ds_dst_slot = node2slot[ds_dst]

    grp_ss = np.bincount((ss_dst_slot // P) * NQ + ss_src_slot % NQ,
                         minlength=nbins_total * NQ).max()
    nch_ss = int(np.ceil(grp_ss / P))

    ss_idx16, ss_lane, ss_glob = _edge_meta(ss_dst_slot, ss_src_slot,
                                            nbins_total, nch_ss, NQ, qsize,
                                            interleaved=True)
    # superbins: groups of 4 consecutive bins (uniform across cores)
    sb_sizes = [4] * (nbins_core // 4)
    ssmeta_sb = []
    for c in range(ncores):
        lob = c * nbins_core
        ssmeta_sb.append(np.stack(_pack_meta_superbins(
            ss_idx16[lob:lob + nbins_core], ss_lane[lob:lob + nbins_core],
            sb_sizes), axis=0))

    # host-precomputed doc->sentence aggregation (layer-invariant):
    # aggdsT[h, slot] = (segment_sum(doc_feat[ds_src], ds_dst)/deg_ds)^T
    aggds = np.zeros((slots_total, H), np.float64)
    np.add.at(aggds, ds_dst_slot, doc_feat[ds_src].astype(np.float64))

    valid = slot2node >= 0
    table0 = np.zeros((slots_total, H), np.float32)
    table0[valid] = s_feat[slot2node[valid]]
    recip_ss = np.ones(slots_total, np.float32)
    recip_ss[valid] = 1.0 / deg_ss[slot2node[valid]]

    deg_ds_slot = np.ones(slots_total, np.float64)
    deg_ds_slot[valid] = deg_ds[slot2node[valid]]
    aggds = (aggds / deg_ds_slot[:, None]).astype(np.float32)

    # layer-0 gather buffer materialized on host: chunk (q,c) of bin b holds
    # table0 rows of its edges in dma_gather layout (edge i -> partition
    # i%128, chunk i//128)
    g0 = table0[ss_glob.reshape(nbins_total, NQ, nch_ss, P)]  # [nb,nq,nch,P,H]
    g0 = np.ascontiguousarray(
        g0.transpose(0, 3, 1, 2, 4).reshape(nbins_total, P, NQ * nch_ss * H))

    iota = np.broadcast_to(np.arange(P, dtype=np.float32)[None, :], (P, P)).copy()
    ident = np.eye(P, dtype=np.float32)

    cfg = Cfg(ncores, nbins_core, ns, nd, nlayers, nch_ss, 0, nq=NQ,
              sb_sizes=sb_sizes)

    in_maps = []
    for c in range(ncores):
        lo, hi = c * slots_core, (c + 1) * slots_core
        lob, hib = c * nbins_core, (c + 1) * nbins_core
        in_maps.append({
            "table0": table0,
            "sT0": np.ascontiguousarray(table0[lo:hi].T),
            "g0": g0[lob:hib],
            "ssmeta": ssmeta_sb[c],
            "aggdsT": np.ascontiguousarray(aggds[lo:hi].T),
            "recipss": np.ascontiguousarray(
                np.broadcast_to(recip_ss[lo:hi][None, :], (P, slots_core))),
            "wr": W_rel,
            "wl": W_loop,
            "biast": bias,
            "iotat": iota,
            "ident": ident,
        })
    return cfg, in_maps, slot2node


def build_program(cfg):
    import concourse.bacc as bacc
    import concourse.mybir as mybir
    import concourse.tile as tile
    from contextlib import ExitStack

    dt = mybir.dt
    f32 = dt.float32
    i32 = dt.int32
    AF = mybir.ActivationFunctionType
    OP = mybir.AluOpType
    L = cfg.L
    NQ, NCH, W = cfg.NQ, cfg.NCH_SS, 4
    NSB = cfg.NBINS // W
    NKB = NQ * NCH            # chunks per bin
    NKSB = NQ * W * NCH       # chunks per superbin gather group

    nc = bacc.Bacc("TRN2", target_bir_lowering=False)

    table0 = nc.dram_tensor("table0", [cfg.SLOTS_TOTAL, H], f32, kind="ExternalInput")
    sT0 = nc.dram_tensor("sT0", [H, cfg.SLOTS_CORE], f32, kind="ExternalInput")
    g0d = nc.dram_tensor("g0", [cfg.NBINS, P, NKB * H], f32, kind="ExternalInput")
    ssmeta = nc.dram_tensor("ssmeta", [NSB, P, 5 * NKSB], i32, kind="ExternalInput")
    aggdsT = nc.dram_tensor("aggdsT", [H, cfg.SLOTS_CORE], f32, kind="ExternalInput")
    recipss = nc.dram_tensor("recipss", [P, cfg.SLOTS_CORE], f32, kind="ExternalInput")
    wr = nc.dram_tensor("wr", [L, 2, H, H], f32, kind="ExternalInput")
    wl = nc.dram_tensor("wl", [L, H, H], f32, kind="ExternalInput")
    biast = nc.dram_tensor("biast", [L, H], f32, kind="ExternalInput")
    iotat = nc.dram_tensor("iotat", [P, P], f32, kind="ExternalInput")
    ident = nc.dram_tensor("ident", [P, P], f32, kind="ExternalInput")
    out_ext = nc.dram_tensor("out", [cfg.SLOTS_CORE, H], f32, kind="ExternalOutput")

    tables = [table0]
    shards = []
    hsT = [sT0]
    for l in range(1, L):
        tables.append(nc.dram_tensor(f"hsf{l}", [cfg.SLOTS_TOTAL, H], f32))
        shards.append(nc.dram_tensor(f"hss{l}", [cfg.SLOTS_CORE, H], f32))
        hsT.append(nc.dram_tensor(f"hsT{l}", [H, cfg.SLOTS_CORE], f32))

    rg = [list(range(cfg.NCORES))]

    with tile.TileContext(nc) as tc, ExitStack() as ctx:
        consts = ctx.enter_context(tc.tile_pool(name="consts", bufs=1))
        meta_p = ctx.enter_context(tc.tile_pool(name="meta", bufs=3))
        gsb_p = ctx.enter_context(tc.tile_pool(name="gsb", bufs=2))
        g0_p = ctx.enter_context(tc.tile_pool(name="g0", bufs=3))
        s_p = ctx.enter_context(tc.tile_pool(name="onehot", bufs=3))
        sm_p = ctx.enter_context(tc.tile_pool(name="small", bufs=4))
        out_p = ctx.enter_context(tc.tile_pool(name="outs", bufs=4))
        ps_agg = ctx.enter_context(tc.tile_pool(name="pagg", bufs=2, space="PSUM"))
        ps_h = ctx.enter_context(tc.tile_pool(name="ph", bufs=2, space="PSUM"))
        ps_t = ctx.enter_context(tc.tile_pool(name="pt", bufs=2, space="PSUM"))

        w0t, w1t, wlt, bt = [], [], [], []
        for l in range(L):
            t = consts.tile([H, H], f32, tag=f"w0_{l}")
            nc.sync.dma_start(t[:], wr[l, 0])
            w0t.append(t)
            t = consts.tile([H, H], f32, tag=f"w1_{l}")
            nc.sync.dma_start(t[:], wr[l, 1])
            w1t.append(t)
            t = consts.tile([H, H], f32, tag=f"wl_{l}")
            nc.sync.dma_start(t[:], wl[l])
            wlt.append(t)
            t = consts.tile([1, H], f32, tag=f"b_{l}")
            nc.sync.dma_start(t[:], biast[l : l + 1, :])
            bt.append(t)
        iota_t = consts.tile([P, P], f32, tag="iota")
        nc.sync.dma_start(iota_t[:], iotat[:])
        ident_t = consts.tile([P, P], f32, tag="ident")
        nc.sync.dma_start(ident_t[:], ident[:])
        ones_t = consts.tile([1, H], f32, tag="ones")
        nc.gpsimd.memset(ones_t[:], 1.0)

        for l in range(L):
            last = l == L - 1
            for sb in range(NSB):
                m = meta_p.tile([P, 5 * NKSB], i32, tag="m")
                nc.sync.dma_start(m[:], ssmeta[sb])
                lanes = m[:, :NKSB].bitcast(f32).rearrange(
                    "p (q w n) -> p q w n", q=NQ, w=W)
                gsb = None
                if l > 0:
                    gsb = gsb_p.tile([P, NKSB * P], f32, tag="gsb")
                    t4 = tables[l][:].rearrange("(r f) h -> r f h", f=NQ)
                    for q in range(NQ):
                        idx16 = m[:, NKSB + q * W * NCH * 4
                                  : NKSB + (q + 1) * W * NCH * 4].bitcast(dt.int16)
                        out3 = gsb[:, q * W * NCH * P : (q + 1) * W * NCH * P
                                   ].rearrange("p (c j) -> p c j", j=P)
                        nc.gpsimd.dma_gather(
                            out_ap=out3,
                            in_ap=t4[:, q, :],
                            idxs_ap=idx16,
                            num_idxs=W * NCH * P, num_idxs_reg=W * NCH * P,
                            elem_size=H, elem_step=NQ * H, single_packet=False)
                for j in range(W):
                    b = sb * W + j
                    if l == 0:
                        g = g0_p.tile([P, NKB * P], f32, tag="g0t")
                        nc.scalar.dma_start(g[:], g0d[b])
                        chunk = lambda k: g[:, k * P : (k + 1) * P]
                    else:
                        chunk = lambda k, _j=j: gsb[
                            :, ((k // NCH) * W * NCH + _j * NCH + (k % NCH)) * P
                            : ((k // NCH) * W * NCH + _j * NCH + (k % NCH)) * P + P]
                    s = s_p.tile([P, NKB * P], f32, tag="s")
                    lanes4 = lanes[:, :, j, :][:, :, :, None].to_broadcast(
                        (P, NQ, NCH, P))
                    iota4 = iota_t[:, None, None, :].to_broadcast((P, NQ, NCH, P))
                    nc.vector.tensor_tensor(
                        out=s[:].rearrange("p (q n j2) -> p q n j2", q=NQ, n=NCH),
                        in0=lanes4, in1=iota4, op=OP.is_equal)
                    pagg = ps_agg.tile([H, P], f32, tag="pagg")
                    for k in range(NKB):
                        nc.tensor.matmul(
                            out=pagg[:], lhsT=chunk(k), rhs=s[:, k * P : (k + 1) * P],
                            start=(k == 0), stop=(k == NKB - 1))
                    r = sm_p.tile([P, P], f32, tag="recip")
                    nc.scalar.dma_start(r[:], recipss[:, b * P : (b + 1) * P])
                    a = sm_p.tile([H, P], f32, tag="aggT")
                    nc.vector.tensor_tensor(out=a[:], in0=pagg[:], in1=r[:], op=OP.mult)

                    ads = sm_p.tile([H, P], f32, tag="ads_in")
                    nc.scalar.dma_start(ads[:], aggdsT[:, b * P : (b + 1) * P])
                    hT_in = sm_p.tile([H, P], f32, tag="hT_in")
                    nc.scalar.dma_start(hT_in[:], hsT[l][:, b * P : (b + 1) * P])
                    ph = ps_h.tile([P, H], f32, tag="ph")
                    nc.tensor.matmul(out=ph[:], lhsT=a[:], rhs=w0t[l][:],
                                     start=True, stop=False)
                    nc.tensor.matmul(out=ph[:], lhsT=ads[:], rhs=w1t[l][:],
                                     start=False, stop=False)
                    nc.tensor.matmul(out=ph[:], lhsT=hT_in[:], rhs=wlt[l][:],
                                     start=False, stop=False)
                    nc.tensor.matmul(out=ph[:], lhsT=ones_t[:], rhs=bt[l][:],
                                     start=False, stop=True)
                    h = out_p.tile([P, H], f32, tag="h_out")
                    nc.scalar.activation(h[:], ph[:], AF.Relu)
                    if last:
                        nc.sync.dma_start(out_ext[b * P : (b + 1) * P, :], h[:])
                    else:
                        nc.sync.dma_start(shards[l][b * P : (b + 1) * P, :], h[:])
                        pt = ps_t.tile([P, H], f32, tag="pt")
                        nc.tensor.transpose(pt[:], h[:], ident_t[:])
                        hT_o = out_p.tile([H, P], f32, tag="hT_out")
                        nc.vector.tensor_copy(hT_o[:], pt[:])
                        nc.scalar.dma_start(hsT[l + 1][:, b * P : (b + 1) * P], hT_o[:])
            if not last:
                nc.gpsimd.collective_compute(
                    "AllGather", mybir.AluOpType.bypass,
                    replica_groups=rg,
                    ins=[shards[l][:]],
                    outs=[tables[l + 1][:]],
                )
    nc.compile()
    return nc


_CACHE = {}


def _run(cfg, in_maps, **kwargs):
    from concourse.bass_utils import run_bass_kernel_spmd

    key = (cfg.NCORES, cfg.NBINS, cfg.NCH_SS, cfg.NCH_DS, cfg.ND, cfg.L)
    if key not in _CACHE:
        _CACHE[key] = build_program(cfg)
    nc = _CACHE[key]
    return run_bass_kernel_spmd(nc, in_maps, list(range(cfg.NCORES)), **kwargs)


def kernel(**inputs) -> np.ndarray:
    cfg, in_maps, slot2node = preprocess(inputs, ncores=8)
    results = _run(cfg, in_maps).results
    ns = inputs["s_feat"].shape[0]
    out = np.zeros((ns, H), np.float32)
    full = np.concatenate([results[c]["out"] for c in range(cfg.NCORES)], axis=0)
    valid = slot2node >= 0
    out[slot2node[valid]] = full[valid]
    return out



# revision 3
# speedup vs baseline: 2.9341x; 2.9341x over previous
"""Trainium2 Bass kernel for 3-layer hetero-GNN message passing (RGCN-style).

Reference semantics (per layer l):
    agg_ss = segment_sum(hs[ss_src], ss_dst) / max(indeg_ss, 1)
    agg_ds = segment_sum(hd[ds_src], ds_dst) / max(indeg_ds, 1)
    hs     = relu(agg_ss @ W_rel[l,0] + agg_ds @ W_rel[l,1] + hs @ W_loop[l] + bias[l])
(doc features hd never change, so agg_ds is layer-invariant.)

Sharding: destination ("sentence") nodes are bin-packed on the host into
128-node bins balanced by in-degree; 100 bins per core x 8 cores.  Edges are
partitioned by destination bin.  Per-relation weights are replicated.  Each
layer ends with an AllGather of the updated node features so every core has
the full "halo" table for the next layer's source gathers.

Perf notes vs the fp32 baseline:
  - all tables / gathered data / one-hots / weights are bf16 (fp32 PSUM)
  - the per-superbin quartile dma_gathers are spread across the 4 SWDGE
    queues with an enlarged descriptor-ring carveout so descriptor
    generation pipelines against SDMA drain instead of serializing
  - recip (in-degree) and the layer-invariant doc aggregation stay resident
    in SBUF; AllGather outputs are Shared-address-space DRAM tensors
"""

import os
import sys
import heapq

import numpy as np

for _p in ("/opt/trn_rl_repo", "/root/.axon_site/_ro/trn_rl_repo"):
    if os.path.isdir(_p) and _p not in sys.path:
        sys.path.insert(0, _p)

import ml_dtypes

BF16 = ml_dtypes.bfloat16

P = 128
H = 128


class Cfg:
    def __init__(self, ncores, nbins_core, ns, nd, nlayers, nch_ss, nch_ds,
                 nq=4, sb_sizes=(), bf16_tables=True):
        self.NCORES = ncores
        self.NBINS = nbins_core              # bins per core
        self.SLOTS_CORE = nbins_core * P
        self.SLOTS_TOTAL = self.SLOTS_CORE * ncores
        self.NS = ns
        self.ND = nd
        self.L = nlayers
        self.NCH_SS = nch_ss
        self.NCH_DS = nch_ds
        self.NQ = nq                         # src quartiles for int16 gather
        self.QSIZE = self.SLOTS_TOTAL // nq
        self.SB = list(sb_sizes)             # superbin widths (all 4)
        self.BF16 = bf16_tables


def pack_bins_q(qvec, cls, nbins, nq=4, cap=P, ncand=6):
    """Quartile-aware LPT: balance per-(bin, src-class) in-edge loads.

    qvec [ns, nq]: node's in-edge count per source class.
    cls  [ns]: node's own class (must get a lane == cls mod nq in its bin).
    Greedy on max class load, with per-bin class capacity cap//nq.
    Returns per-bin node lists (each list entry (node, cls))."""
    tot = qvec.sum(1)
    order = np.argsort(-tot, kind="stable")
    ccap = cap // nq
    counts = np.zeros(nbins, np.int64)
    ccounts = np.zeros((nbins, nq), np.int64)
    qloads = np.zeros((nbins, nq), np.int64)
    key = np.zeros(nbins, np.int64)
    bins = [[] for _ in range(nbins)]
    heap = [(0, b) for b in range(nbins)]
    heapq.heapify(heap)
    for n in order:
        c = int(cls[n])
        v = qvec[n]
        cands, stash = [], []
        while heap and len(cands) < ncand:
            k, b = heapq.heappop(heap)
            if k != key[b] or counts[b] >= cap:
                continue  # stale or permanently full
            if ccounts[b, c] >= ccap:
                stash.append((k, b))  # full for this class only
                continue
            cands.append(b)
        assert cands, "no bin with free class slot"
        best = min(cands, key=lambda b: (int(np.max(qloads[b] + v)),
                                         int(qloads[b].sum())))
        bins[best].append(int(n))
        counts[best] += 1
        ccounts[best, c] += 1
        qloads[best] += v
        key[best] = int(qloads[best].max())
        for k, b in stash:
            heapq.heappush(heap, (k, b))
        for b in cands:
            if counts[b] < cap:
                heapq.heappush(heap, (int(key[b]), b))
    return bins, qloads


def _edge_meta(dst_slot, src_idx, nbins_total, nch, nq, qsize,
               interleaved=False):
    """Per-(bin, src-quartile) edge arrays for dma_gather.

    Edges of a bin are grouped by source quartile (interleaved: q = src % nq,
    local row = src // nq; else q = src // qsize, local = src % qsize);
    within a group edge k lands at partition k%128, chunk k//128 (dma_gather
    layout).
    Returns (idx16, lane, glob):
      idx16 [nbins, nq, 128, nch*8] int16  -- local row id, wrapped
          16-partition layout replicated to 128 partitions; pad 0
      lane  [nbins, nq, 128, nch] float32  -- dst lane in bin; pad -1
      glob  [nbins, nq, nch*128] int64     -- global src row id (pads map to
          a valid row of the right quartile)
    """
    if interleaved:
        q_of_edge = src_idx % nq
        src_local = (src_idx // nq).astype(np.int16)
    else:
        q_of_edge = src_idx // qsize
        src_local = (src_idx % qsize).astype(np.int16)
    bin_of_edge = dst_slot // P
    lane_of_edge = (dst_slot % P).astype(np.float32)
    group = bin_of_edge * nq + q_of_edge
    order = np.argsort(group, kind="stable")
    g_sorted = group[order]
    src_sorted = src_local[order]
    lane_sorted = lane_of_edge[order]
    ngroups = nbins_total * nq
    counts = np.bincount(g_sorted, minlength=ngroups)
    starts = np.concatenate([[0], np.cumsum(counts)[:-1]])
    k_in_g = np.arange(len(order)) - starts[g_sorted]
    assert k_in_g.max(initial=0) < nch * P, (k_in_g.max(initial=0), nch * P)
    pp = (k_in_g % P).astype(np.int64)
    cc = (k_in_g // P).astype(np.int64)

    lane = np.full((ngroups, P, nch), -1.0, np.float32)
    lane[g_sorted, pp, cc] = lane_sorted

    # idx in dma_gather wrapped layout: idx of edge i sits at
    # [i % 16, i // 16] of a [16, nch*8] array, replicated to 128 partitions.
    idx_flat = np.zeros((ngroups, nch * P), np.int16)
    idx_flat[g_sorted, k_in_g] = src_sorted
    glob = idx_flat.astype(np.int64).reshape(nbins_total, nq, nch * P)
    qbase = np.arange(nq, dtype=np.int64)
    if interleaved:
        glob = glob * nq + qbase[None, :, None]
    else:
        glob = glob + (qbase * qsize)[None, :, None]
    idx16 = idx_flat.reshape(ngroups, nch * 8, 16).transpose(0, 2, 1)
    idx16 = np.broadcast_to(idx16[:, None, :, :], (ngroups, 8, 16, nch * 8))
    idx16 = idx16.reshape(ngroups, P, nch * 8)
    return (np.ascontiguousarray(idx16.reshape(nbins_total, nq, P, nch * 8)),
            np.ascontiguousarray(lane.reshape(nbins_total, nq, P, nch)),
            glob)


def _pack_meta_superbins(idx16, lane, sb_sizes):
    """Pack meta per superbin (group of consecutive bins).

    For a superbin of w bins: per quartile the w bins' idx streams concatenate
    (each bin is a whole number of 128-edge chunks, so wrapped int16 layouts
    concatenate along the free axis).

    Returns list over superbins of int32 arrays
      [P, nq*w*nch//2 (lanes bf16) + nq*w*nch*4 (idx16)]
    """
    nbins, nq, _, nch8 = idx16.shape
    nch = nch8 // 8
    out = []
    b0 = 0
    for w in sb_sizes:
        # lanes: [w, nq, P, nch] -> [P, nq, w, nch] -> [P, nq*w*nch] bf16
        lane_sb = lane[b0:b0 + w].transpose(2, 1, 0, 3).reshape(P, nq * w * nch)
        lane_bf = np.ascontiguousarray(lane_sb).astype(BF16)
        assert (nq * w * nch) % 2 == 0
        lane_i32 = np.ascontiguousarray(lane_bf).view(np.int32)
        # idx16: [w, nq, P, nch8] -> [P, nq, w, nch8] -> [P, nq*w*nch8] int16
        idx_sb = idx16[b0:b0 + w].transpose(2, 1, 0, 3).reshape(P, nq * w * nch8)
        idx_i32 = np.ascontiguousarray(idx_sb).view(np.int32)
        out.append(np.ascontiguousarray(
            np.concatenate([lane_i32, idx_i32], axis=1)))
        b0 += w
    return out


def preprocess(inputs, ncores=8, nbins_core=None):
    """Host-side graph partitioning.  Returns (cfg, per-core input maps,
    slot2node) -- slot2node maps device output rows back to node ids."""
    s_feat = np.asarray(inputs["s_feat"], np.float32)
    doc_feat = np.asarray(inputs["doc_feat"], np.float32)
    W_rel = np.asarray(inputs["W_rel"], np.float32)
    W_loop = np.asarray(inputs["W_loop"], np.float32)
    bias = np.asarray(inputs["bias"], np.float32)
    ss_src = np.asarray(inputs["ss_src"], np.int64)
    ss_dst = np.asarray(inputs["ss_dst"], np.int64)
    ds_src = np.asarray(inputs["ds_src"], np.int64)
    ds_dst = np.asarray(inputs["ds_dst"], np.int64)

    ns, h = s_feat.shape
    nd = doc_feat.shape[0]
    nlayers = W_loop.shape[0]
    assert h == H

    if nbins_core is None:
        nbins_core = int(np.ceil(ns / (ncores * P)))
    nbins_core = (nbins_core + 3) // 4 * 4  # whole superbins of 4
    nbins_total = nbins_core * ncores
    slots_core = nbins_core * P
    slots_total = nbins_total * P

    cnt_ss = np.bincount(ss_dst, minlength=ns)
    cnt_ds = np.bincount(ds_dst, minlength=ns)
    deg_ss = np.maximum(cnt_ss, 1).astype(np.float32)
    deg_ds = np.maximum(cnt_ds, 1).astype(np.float32)

    # src quartiles: int16 gather indices need local row < 32768.
    # quartile = slot % 4 (interleaved sub-tables via elem_step), and a
    # node's class (= its slot % 4) is fixed upfront as node_id % 4 so the
    # packer can balance per-(bin, src-class) edge loads directly.
    NQ = 4
    qsize = slots_total // NQ
    assert slots_total % NQ == 0 and qsize <= 32767

    cls = (np.arange(ns) % NQ).astype(np.int64)
    qvec = np.zeros((ns, NQ), np.int64)
    np.add.at(qvec, (ss_dst, cls[ss_src]), 1)
    bins, _qloads = pack_bins_q(qvec, cls, nbins_total, nq=NQ)

    slot2node = np.full(slots_total, -1, np.int64)
    node2slot = np.full(ns, -1, np.int64)
    for b, nodes in enumerate(bins):
        nxt = [r for r in range(NQ)]
        for n in nodes:
            r = int(cls[n])
            lane_i = nxt[r]
            nxt[r] += NQ
            s = b * P + lane_i
            slot2node[s] = n
            node2slot[n] = s
    assert (node2slot >= 0).all()

    ss_dst_slot = node2slot[ss_dst]
    ss_src_slot = node2slot[ss_src]
    ds_dst_slot = node2slot[ds_dst]

    grp_ss = np.bincount((ss_dst_slot // P) * NQ + ss_src_slot % NQ,
                         minlength=nbins_total * NQ).max()
    nch_ss = int(np.ceil(grp_ss / P))

    ss_idx16, ss_lane, ss_glob = _edge_meta(ss_dst_slot, ss_src_slot,
                                            nbins_total, nch_ss, NQ, qsize,
                                            interleaved=True)
    # superbins: groups of 4 consecutive bins (uniform across cores)
    sb_sizes = [4] * (nbins_core // 4)
    ssmeta_sb = []
    for c in range(ncores):
        lob = c * nbins_core
        ssmeta_sb.append(np.stack(_pack_meta_superbins(
            ss_idx16[lob:lob + nbins_core], ss_lane[lob:lob + nbins_core],
            sb_sizes), axis=0))

    # host-precomputed doc->sentence aggregation (layer-invariant):
    # aggdsT[h, slot] = (segment_sum(doc_feat[ds_src], ds_dst)/deg_ds)^T
    aggds = np.zeros((slots_total, H), np.float64)
    np.add.at(aggds, ds_dst_slot, doc_feat[ds_src].astype(np.float64))

    valid = slot2node >= 0
    table0 = np.zeros((slots_total, H), BF16)
    table0[valid] = s_feat[slot2node[valid]].astype(BF16)
    recip_ss = np.ones(slots_total, np.float32)
    recip_ss[valid] = 1.0 / deg_ss[slot2node[valid]]

    deg_ds_slot = np.ones(slots_total, np.float64)
    deg_ds_slot[valid] = deg_ds[slot2node[valid]]
    aggds = (aggds / deg_ds_slot[:, None]).astype(BF16)

    # layer-0 gather buffer materialized on host: chunk (q,c) of bin b holds
    # table0 rows of its edges in dma_gather layout (edge i -> partition
    # i%128, chunk i//128)
    g0 = table0[ss_glob.reshape(nbins_total, NQ, nch_ss, P)]  # [nb,nq,nch,P,H]
    g0 = np.ascontiguousarray(
        g0.transpose(0, 3, 1, 2, 4).reshape(nbins_total, P, NQ * nch_ss * H))

    iota = np.broadcast_to(
        np.arange(P, dtype=np.float32)[None, :], (P, P)).astype(BF16)
    ident = np.eye(P, dtype=np.float32).astype(BF16)

    cfg = Cfg(ncores, nbins_core, ns, nd, nlayers, nch_ss, 0, nq=NQ,
              sb_sizes=sb_sizes)

    W_rel_bf = W_rel.astype(BF16)
    W_loop_bf = W_loop.astype(BF16)
    bias_bf = bias.astype(BF16)

    in_maps = []
    for c in range(ncores):
        lo, hi = c * slots_core, (c + 1) * slots_core
        lob, hib = c * nbins_core, (c + 1) * nbins_core
        in_maps.append({
            "table0": table0,
            "sT0": np.ascontiguousarray(table0[lo:hi].T),
            "g0": g0[lob:hib],
            "ssmeta": ssmeta_sb[c],
            "aggdsT": np.ascontiguousarray(aggds[lo:hi].T),
            "recipss": np.ascontiguousarray(
                np.broadcast_to(recip_ss[lo:hi][None, :], (P, slots_core))),
            "wr": W_rel_bf,
            "wl": W_loop_bf,
            "biast": bias_bf,
            "iotat": np.ascontiguousarray(iota),
            "ident": np.ascontiguousarray(ident),
        })
    return cfg, in_maps, slot2node


def build_program(cfg):
    import concourse.bacc as bacc
    import concourse.mybir as mybir
    import concourse.tile as tile
    from contextlib import ExitStack

    dt = mybir.dt
    f32 = dt.float32
    bf16 = dt.bfloat16
    i32 = dt.int32
    AF = mybir.ActivationFunctionType
    OP = mybir.AluOpType
    L = cfg.L
    NQ, NCH, W = cfg.NQ, cfg.NCH_SS, 4
    NSB = cfg.NBINS // W
    NKB = NQ * NCH            # chunks per bin
    NKSB = NQ * W * NCH       # chunks per superbin gather group

    nc = bacc.Bacc("TRN2", target_bir_lowering=False,
                   num_swdge_queues=4, dynamic_dma_scratch_size=49152)

    table0 = nc.dram_tensor("table0", [cfg.SLOTS_TOTAL, H], bf16, kind="ExternalInput")
    sT0 = nc.dram_tensor("sT0", [H, cfg.SLOTS_CORE], bf16, kind="ExternalInput")
    g0d = nc.dram_tensor("g0", [cfg.NBINS, P, NKB * H], bf16, kind="ExternalInput")
    assert NKSB % 2 == 0
    ssmeta = nc.dram_tensor("ssmeta", [NSB, P, NKSB // 2 + 4 * NKSB], i32, kind="ExternalInput")
    aggdsT = nc.dram_tensor("aggdsT", [H, cfg.SLOTS_CORE], bf16, kind="ExternalInput")
    recipss = nc.dram_tensor("recipss", [P, cfg.SLOTS_CORE], f32, kind="ExternalInput")
    wr = nc.dram_tensor("wr", [L, 2, H, H], bf16, kind="ExternalInput")
    wl = nc.dram_tensor("wl", [L, H, H], bf16, kind="ExternalInput")
    biast = nc.dram_tensor("biast", [L, H], bf16, kind="ExternalInput")
    iotat = nc.dram_tensor("iotat", [P, P], bf16, kind="ExternalInput")
    ident = nc.dram_tensor("ident", [P, P], bf16, kind="ExternalInput")
    out_ext = nc.dram_tensor("out", [cfg.SLOTS_CORE, H], f32, kind="ExternalOutput")

    tables = [table0]
    shards = []
    hsT = [sT0]
    for l in range(1, L):
        tables.append(nc.dram_tensor(f"hsf{l}", [cfg.SLOTS_TOTAL, H], bf16,
                                     addr_space="Shared"))
        shards.append(nc.dram_tensor(f"hss{l}", [cfg.SLOTS_CORE, H], bf16))
        hsT.append(nc.dram_tensor(f"hsT{l}", [H, cfg.SLOTS_CORE], bf16))

    rg = [list(range(cfg.NCORES))]

    with tile.TileContext(nc) as tc, ExitStack() as ctx:
        consts = ctx.enter_context(tc.tile_pool(name="consts", bufs=1))
        meta_p = ctx.enter_context(tc.tile_pool(name="meta", bufs=3))
        gsb_p = ctx.enter_context(tc.tile_pool(name="gsb", bufs=2))
        g0_p = ctx.enter_context(tc.tile_pool(name="g0", bufs=3))
        s_p = ctx.enter_context(tc.tile_pool(name="onehot", bufs=3))
        sm_p = ctx.enter_context(tc.tile_pool(name="small", bufs=4))
        out_p = ctx.enter_context(tc.tile_pool(name="outs", bufs=4))
        ps_agg = ctx.enter_context(tc.tile_pool(name="pagg", bufs=2, space="PSUM"))
        ps_h = ctx.enter_context(tc.tile_pool(name="ph", bufs=2, space="PSUM"))
        ps_t = ctx.enter_context(tc.tile_pool(name="pt", bufs=2, space="PSUM"))

        w0t, w1t, wlt, bt = [], [], [], []
        for l in range(L):
            t = consts.tile([H, H], bf16, tag=f"w0_{l}")
            nc.sync.dma_start(t[:], wr[l, 0])
            w0t.append(t)
            t = consts.tile([H, H], bf16, tag=f"w1_{l}")
            nc.sync.dma_start(t[:], wr[l, 1])
            w1t.append(t)
            t = consts.tile([H, H], bf16, tag=f"wl_{l}")
            nc.sync.dma_start(t[:], wl[l])
            wlt.append(t)
            t = consts.tile([1, H], bf16, tag=f"b_{l}")
            nc.sync.dma_start(t[:], biast[l : l + 1, :])
            bt.append(t)
        iota_t = consts.tile([P, P], bf16, tag="iota")
        nc.sync.dma_start(iota_t[:], iotat[:])
        ident_t = consts.tile([P, P], bf16, tag="ident")
        nc.sync.dma_start(ident_t[:], ident[:])
        ones_t = consts.tile([1, H], bf16, tag="ones")
        nc.gpsimd.memset(ones_t[:], 1.0)
        # layer-invariant residents: 1/deg (fp32) and doc-relation agg (bf16)
        recip_t = consts.tile([P, cfg.SLOTS_CORE], f32, tag="recip")
        nc.sync.dma_start(recip_t[:], recipss[:])
        aggds_t = consts.tile([H, cfg.SLOTS_CORE], bf16, tag="aggds")
        nc.sync.dma_start(aggds_t[:], aggdsT[:])

        for l in range(L):
            last = l == L - 1
            for sb in range(NSB):
                m = meta_p.tile([P, NKSB // 2 + 4 * NKSB], i32, tag="m")
                nc.sync.dma_start(m[:], ssmeta[sb])
                lanes = m[:, :NKSB // 2].bitcast(bf16).rearrange(
                    "p (q w n) -> p q w n", q=NQ, w=W)
                gsb = None
                if l > 0:
                    gsb = gsb_p.tile([P, NKSB * P], bf16, tag="gsb")
                    t4 = tables[l][:].rearrange("(r f) h -> r f h", f=NQ)
                    for q in range(NQ):
                        idx16 = m[:, NKSB // 2 + q * W * NCH * 4
                                  : NKSB // 2 + (q + 1) * W * NCH * 4].bitcast(dt.int16)
                        out3 = gsb[:, q * W * NCH * P : (q + 1) * W * NCH * P
                                   ].rearrange("p (c j) -> p c j", j=P)
                        nc.gpsimd.dma_gather(
                            out_ap=out3,
                            in_ap=t4[:, q, :],
                            idxs_ap=idx16,
                            num_idxs=W * NCH * P, num_idxs_reg=W * NCH * P,
                            elem_size=H, elem_step=NQ * H, single_packet=False,
                            queue_num=q)
                for j in range(W):
                    b = sb * W + j
                    if l == 0:
                        g = g0_p.tile([P, NKB * P], bf16, tag="g0t")
                        nc.scalar.dma_start(g[:], g0d[b])
                        chunk = lambda k: g[:, k * P : (k + 1) * P]
                    else:
                        chunk = lambda k, _j=j: gsb[
                            :, ((k // NCH) * W * NCH + _j * NCH + (k % NCH)) * P
                            : ((k // NCH) * W * NCH + _j * NCH + (k % NCH)) * P + P]
                    s = s_p.tile([P, NKB * P], bf16, tag="s")
                    lanes4 = lanes[:, :, j, :][:, :, :, None].to_broadcast(
                        (P, NQ, NCH, P))
                    iota4 = iota_t[:, None, None, :].to_broadcast((P, NQ, NCH, P))
                    nc.vector.tensor_tensor(
                        out=s[:].rearrange("p (q n j2) -> p q n j2", q=NQ, n=NCH),
                        in0=lanes4, in1=iota4, op=OP.is_equal)
                    pagg = ps_agg.tile([H, P], f32, tag="pagg")
                    for k in range(NKB):
                        nc.tensor.matmul(
                            out=pagg[:], lhsT=chunk(k), rhs=s[:, k * P : (k + 1) * P],
                            start=(k == 0), stop=(k == NKB - 1))
                    a = sm_p.tile([H, P], bf16, tag="aggT")
                    nc.vector.tensor_tensor(
                        out=a[:], in0=pagg[:],
                        in1=recip_t[:, b * P : (b + 1) * P], op=OP.mult)

                    hT_in = sm_p.tile([H, P], bf16, tag="hT_in")
                    nc.scalar.dma_start(hT_in[:], hsT[l][:, b * P : (b + 1) * P])
                    ph = ps_h.tile([P, H], f32, tag="ph")
                    nc.tensor.matmul(out=ph[:], lhsT=a[:], rhs=w0t[l][:],
                                     start=True, stop=False)
                    nc.tensor.matmul(out=ph[:],
                                     lhsT=aggds_t[:, b * P : (b + 1) * P],
                                     rhs=w1t[l][:], start=False, stop=False)
                    nc.tensor.matmul(out=ph[:], lhsT=hT_in[:], rhs=wlt[l][:],
                                     start=False, stop=False)
                    nc.tensor.matmul(out=ph[:], lhsT=ones_t[:], rhs=bt[l][:],
                                     start=False, stop=True)
                    if last:
                        h = out_p.tile([P, H], f32, tag="h_out32")
                        nc.scalar.activation(h[:], ph[:], AF.Relu)
                        nc.sync.dma_start(out_ext[b * P : (b + 1) * P, :], h[:])
                    else:
                        h = out_p.tile([P, H], bf16, tag="h_out")
                        nc.scalar.activation(h[:], ph[:], AF.Relu)
                        nc.sync.dma_start(shards[l][b * P : (b + 1) * P, :], h[:])
                        pt = ps_t.tile([P, H], bf16, tag="pt")
                        nc.tensor.transpose(pt[:], h[:], ident_t[:])
                        hT_o = out_p.tile([H, P], bf16, tag="hT_out")
                        nc.vector.tensor_copy(hT_o[:], pt[:])
                        nc.scalar.dma_start(hsT[l + 1][:, b * P : (b + 1) * P], hT_o[:])
            if not last:
                nc.gpsimd.collective_compute(
                    "AllGather", mybir.AluOpType.bypass,
                    replica_groups=rg,
                    ins=[shards[l][:]],
                    outs=[tables[l + 1][:]],
                )
    nc.compile()
    return nc


_CACHE = {}


def _run(cfg, in_maps, **kwargs):
    from concourse.bass_utils import run_bass_kernel_spmd

    key = (cfg.NCORES, cfg.NBINS, cfg.NCH_SS, cfg.NCH_DS, cfg.ND, cfg.L)
    if key not in _CACHE:
        _CACHE[key] = build_program(cfg)
    nc = _CACHE[key]
    return run_bass_kernel_spmd(nc, in_maps, list(range(cfg.NCORES)), **kwargs)


def kernel(**inputs) -> np.ndarray:
    cfg, in_maps, slot2node = preprocess(inputs, ncores=8)
    results = _run(cfg, in_maps).results
    ns = inputs["s_feat"].shape[0]
    out = np.zeros((ns, H), np.float32)
    full = np.concatenate([results[c]["out"] for c in range(cfg.NCORES)], axis=0)
    valid = slot2node >= 0
    out[slot2node[valid]] = full[valid]
    return out
